# revision 5
# baseline (speedup 1.0000x reference)
"""Trainium2 Bass kernel for nn_ConditionalStudentTeacherVGAE (topk_masking).

Pipeline implemented (matches reference.py semantics):
  1. labels = argmax(label_probs, 1); homophily rebalance reweights
     same-class pairs by w_same and diff-class by w_diff (both saturate at
     the clip bounds 3 and 1/3 for generic inputs), symmetrizes, rescales
     by a positive scalar, zeroes the diagonal.
  2. Global top-K (K=131072) over the strict upper triangle -> symmetric
     binary adjacency.
  3. 10 rounds of per-node degree capping at MAX_DEGREE=64 (a no-op
     whenever every degree after step 2 is <= 64 - verified on device).

Key facts exploited (each verified at runtime, with an exact fallback):
  - The post-rebalance value ordering equals the ordering of
    u_ij = fl(w*p_ij) + fl(w*p_ji) (the 0.5 and the mean-ratio scalars are
    positive rescalings - order preserving; bit-exact f32 ops match the
    reference's elementwise computation).
  - diff-class u <= 2*fl(1/3) ~= 0.667 while the top-K threshold is far
    above it, so only same-class pairs can be edges. Sharding nodes by
    class (2 classes per core) makes every candidate edge core-local.
  - Per-row top-64 extraction (vector-engine max8 + match_replace) yields
    a candidate superset of all edges whenever max degree <= 64; the host
    merges 8x[1280,64] candidates and picks the 2K-th largest as the
    threshold. Device then emits the thresholded adjacency + row degrees.
    Host checks sum(deg) == 2K, max(deg) <= 64, t > 0.68: these jointly
    prove the emitted edge set equals the reference's top-K selection and
    that the degree-cap loop is an identity. Any violation -> exact
    jax fallback.

Sharding: 16 classes -> 2 per core. Per-core shard [1280, 8704] f32:
rows = the core's class members (each class padded to S=640), cols =
[own 2-class block (1280) | all other nodes packed (<=7424)], zero pad.
A second input carries the transposed own-class diagonal blocks.
"""

import os
import sys
import types

import numpy as np

# ---------------------------------------------------------------- constants
N = 8192
C = 16
K = 131072
MAXD = 64
S = 640                 # per-class slot (max class size must be <= S)
B = 2 * S               # own-block width / padded rows per core
W_REST = 7424           # packed "other nodes" region (max 8192-855 -> 7337)
W = B + W_REST          # 8704
NT = B // 128           # 10 row tiles per core
N_CORES = 8
F32 = np.float32
W_SAME = F32(3.0)
W_DIFF = F32(1.0 / 3.0)
DIFF_U_MAX = 0.68       # any u above this must be a same-class pair
NEG_FILL = -1.0


# ------------------------------------------------------- axon profile hook
def _install_axon_hook():
    """Register the NTFF profile hook missing from this image so that
    run_bass_kernel_spmd(trace=True) works (and BASS_TRACE=1 doesn't
    crash). Safe no-op if already present."""
    if "antenv.axon_hooks" in sys.modules:
        return
    try:
        mod = types.ModuleType("antenv.axon_hooks")
        _state = {"hook": None}
        mod.set_axon_ntff_profile_hook = lambda h: _state.__setitem__("hook", h)
        mod.get_axon_ntff_profile_hook = lambda: _state["hook"]
        import antenv

        sys.modules["antenv.axon_hooks"] = mod
        antenv.axon_hooks = mod
        try:
            from trn_agent_boot.trn_boot import _ntff_profile_via_ctypes

            so = "/opt/axon/libaxon_pjrt.so"
            if os.path.exists(so):
                mod.set_axon_ntff_profile_hook(_ntff_profile_via_ctypes(so))
        except Exception:
            pass
        try:
            from concourse import bass_utils

            bass_utils.upload_artifacts = lambda tmpdir: tmpdir
        except Exception:
            pass
    except Exception:
        pass


# ------------------------------------------------------------ bass programs
_programs = {}


def _build_pass_a():
    import concourse.tile as tile
    from concourse import bacc, mybir

    dt = mybir.dt
    nc = bacc.Bacc("TRN2", target_bir_lowering=False, debug=False,
                   num_devices=N_CORES)
    shard = nc.dram_tensor("shard", [B, W], dt.float32,
                           kind="ExternalInput").ap()
    blkt = nc.dram_tensor("blkt", [B, S], dt.float32,
                          kind="ExternalInput").ap()
    cand_o = nc.dram_tensor("cand", [B, 64], dt.float32,
                            kind="ExternalOutput").ap()
    sums_o = nc.dram_tensor("sums", [B, 4], dt.float32,
                            kind="ExternalOutput").ap()
    u_o = nc.dram_tensor("ublk", [B, S], dt.float32,
                         kind="ExternalOutput").ap()

    with tile.TileContext(nc) as tc:
        with (
            tc.tile_pool(name="xb", bufs=3) as xb_pool,
            tc.tile_pool(name="xr", bufs=2) as xr_pool,
            tc.tile_pool(name="xt", bufs=3) as xt_pool,
            tc.tile_pool(name="wk", bufs=2) as wk_pool,
            tc.tile_pool(name="cd", bufs=3) as cd_pool,
            tc.tile_pool(name="sm", bufs=3) as sm_pool,
            tc.tile_pool(name="dg", bufs=2) as dg_pool,
        ):
            for ti in range(NT):
                r0 = ti * 128
                own = 0 if ti < NT // 2 else S
                cross = S - own
                dloc = r0 - own  # diag chunk offset inside the own slot

                xb = xb_pool.tile([128, B], dt.float32, tag="xb")
                nc.sync.dma_start(xb[:], shard[r0:r0 + 128, 0:B])
                xr = xr_pool.tile([128, W_REST], dt.float32, tag="xr")
                nc.sync.dma_start(xr[:], shard[r0:r0 + 128, B:W])
                xt = xt_pool.tile([128, S], dt.float32, tag="xt")
                nc.sync.dma_start(xt[:], blkt[r0:r0 + 128, 0:S])

                sm = sm_pool.tile([128, 4], dt.float32, tag="sm")

                # rest-region row sums on the scalar (ACT) engine
                nc.scalar.activation(
                    out=xr[:], in_=xr[:],
                    func=mybir.ActivationFunctionType.Copy,
                    accum_out=sm[:, 0:1],
                )
                # cross-quadrant row sums (raw p)
                nc.vector.tensor_reduce(
                    out=sm[:, 1:2], in_=xb[:, cross:cross + S],
                    axis=mybir.AxisListType.X, op=mybir.AluOpType.add,
                )
                # diagonal extraction (raw p) before in-place scaling
                dgt = dg_pool.tile([128, 128], dt.float32, tag="dg")
                nc.gpsimd.affine_select(
                    out=dgt[:], in_=xb[:, r0:r0 + 128],
                    compare_op=mybir.AluOpType.is_equal, fill=0.0,
                    base=0, pattern=[[-1, 128]], channel_multiplier=1,
                )
                nc.vector.tensor_reduce(
                    out=sm[:, 3:4], in_=dgt[:],
                    axis=mybir.AxisListType.X, op=mybir.AluOpType.add,
                )
                # own-quadrant row sums (raw p), then scale by 3 in place
                xo = xb[:, own:own + S]
                nc.vector.tensor_reduce(
                    out=sm[:, 2:3], in_=xo,
                    axis=mybir.AxisListType.X, op=mybir.AluOpType.add,
                )
                nc.vector.tensor_scalar(
                    out=xo, in0=xo, scalar1=float(W_SAME), scalar2=None,
                    op0=mybir.AluOpType.mult,
                )
                nc.vector.tensor_scalar(
                    out=xt[:], in0=xt[:], scalar1=float(W_SAME), scalar2=None,
                    op0=mybir.AluOpType.mult,
                )
                # u_own = fl(3 p) + fl(3 p^T)
                nc.vector.tensor_tensor(out=xt[:], in0=xo, in1=xt[:],
                                        op=mybir.AluOpType.add)
                # mask the diagonal chunk with -1
                nc.gpsimd.affine_select(
                    out=xt[:, dloc:dloc + 128], in_=xt[:, dloc:dloc + 128],
                    compare_op=mybir.AluOpType.not_equal, fill=NEG_FILL,
                    base=0, pattern=[[-1, 128]], channel_multiplier=1,
                )
                nc.sync.dma_start(u_o[r0:r0 + 128, 0:S], xt[:])
                nc.sync.dma_start(sums_o[r0:r0 + 128, 0:4], sm[:])

                # per-row top-64 extraction
                cand = cd_pool.tile([128, 64], dt.float32, tag="cd")
                w0 = wk_pool.tile([128, S], dt.float32, tag="w0")
                w1 = wk_pool.tile([128, S], dt.float32, tag="w1")
                src = xt[:]
                for r in range(8):
                    c8 = cand[:, r * 8:(r + 1) * 8]
                    nc.vector.max(c8, src)
                    if r < 7:
                        dst = w0[:] if r % 2 == 0 else w1[:]
                        nc.vector.match_replace(dst, c8, src, NEG_FILL)
                        src = dst
                nc.sync.dma_start(cand_o[r0:r0 + 128, 0:64], cand[:])

    nc.compile()
    return nc


def _build_pass_c():
    import concourse.tile as tile
    from concourse import bacc, mybir

    dt = mybir.dt
    nc = bacc.Bacc("TRN2", target_bir_lowering=False, debug=False,
                   num_devices=N_CORES)
    u_i = nc.dram_tensor("ublk", [B, S], dt.float32,
                         kind="ExternalInput").ap()
    t_i = nc.dram_tensor("thr", [128, 1], dt.float32,
                         kind="ExternalInput").ap()
    out_o = nc.dram_tensor("oshard", [B, W], dt.float32,
                           kind="ExternalOutput").ap()
    deg_o = nc.dram_tensor("deg", [B, 1], dt.float32,
                           kind="ExternalOutput").ap()

    with tile.TileContext(nc) as tc:
        with (
            tc.tile_pool(name="ut", bufs=3) as ut_pool,
            tc.tile_pool(name="aj", bufs=3) as aj_pool,
            tc.tile_pool(name="zz", bufs=1) as zz_pool,
            tc.tile_pool(name="tt", bufs=1) as tt_pool,
            tc.tile_pool(name="dg", bufs=3) as dg_pool,
        ):
            tt = tt_pool.tile([128, 1], dt.float32, tag="tt")
            nc.sync.dma_start(tt[:], t_i[0:128, 0:1])
            zz = zz_pool.tile([128, W_REST + S], dt.float32, tag="zz")
            nc.vector.memset(zz[:], 0.0)

            for ti in range(NT):
                r0 = ti * 128
                own = 0 if ti < NT // 2 else S
                ut = ut_pool.tile([128, S], dt.float32, tag="ut")
                nc.sync.dma_start(ut[:], u_i[r0:r0 + 128, 0:S])
                aj = aj_pool.tile([128, S], dt.float32, tag="aj")
                nc.vector.tensor_scalar(
                    out=aj[:], in0=ut[:], scalar1=tt[:, 0:1], scalar2=None,
                    op0=mybir.AluOpType.is_ge,
                )
                dg = dg_pool.tile([128, 1], dt.float32, tag="dgc")
                nc.vector.tensor_reduce(
                    out=dg[:], in_=aj[:],
                    axis=mybir.AxisListType.X, op=mybir.AluOpType.add,
                )
                nc.sync.dma_start(deg_o[r0:r0 + 128, 0:1], dg[:])
                nc.sync.dma_start(out_o[r0:r0 + 128, own:own + S], aj[:])
                if ti < NT // 2:
                    nc.sync.dma_start(out_o[r0:r0 + 128, S:W],
                                      zz[:, 0:W - S])
                else:
                    nc.sync.dma_start(out_o[r0:r0 + 128, 0:S], zz[:, 0:S])
                    nc.sync.dma_start(out_o[r0:r0 + 128, B:W],
                                      zz[:, 0:W_REST])

    nc.compile()
    return nc


def _get_programs():
    if "a" not in _programs:
        _install_axon_hook()
        _programs["a"] = _build_pass_a()
        _programs["c"] = _build_pass_c()
    return _programs["a"], _programs["c"]


# ------------------------------------------------------------ exact fallback
def _reference_fallback(adj_probs, label_probs):
    """Bit-faithful replication of reference.py (used only if a fast-path
    validity check fails)."""
    import jax
    import jax.numpy as jnp

    TARGET_EDGES, MAX_DEGREE = K, MAXD
    TARGET_HOM, CLAMP, TOL, EPS, MAX_ITERS = 0.7, 3.0, 0.05, 1e-6, 10
    adj_probs = jnp.asarray(adj_probs)
    label_probs = jnp.asarray(label_probs)
    eye = jnp.eye(N, dtype=adj_probs.dtype)
    labels = jnp.argmax(label_probs, axis=1)
    same = (labels[:, None] == labels[None, :]).astype(adj_probs.dtype) * (1.0 - eye)
    diff = 1.0 - same
    same_mass = jnp.sum(adj_probs * same)
    diff_mass = jnp.sum(adj_probs * diff)
    total = same_mass + diff_mass
    cur = same_mass / total
    w_same = jnp.clip(TARGET_HOM / (cur + EPS), 1.0 / CLAMP, CLAMP)
    w_diff = jnp.clip((1.0 - TARGET_HOM) / ((1.0 - cur) + EPS), 1.0 / CLAMP, CLAMP)
    weights = same * w_same + diff * w_diff
    scaled = adj_probs * weights
    scaled = 0.5 * (scaled + scaled.T)
    scaled = scaled * (jnp.mean(adj_probs) / jnp.maximum(jnp.mean(scaled), EPS))
    scaled = jnp.maximum(scaled, 0.0) * (1.0 - eye)
    skip = (same_mass <= EPS) | (diff_mass <= EPS) | (jnp.abs(cur - TARGET_HOM) <= TOL)
    probs = jnp.where(skip, adj_probs, scaled)

    upper_mask = jnp.triu(jnp.ones((N, N), dtype=bool), k=1)
    neg_inf = jnp.array(-jnp.inf, dtype=probs.dtype)
    flat = jnp.where(upper_mask, probs, neg_inf).reshape(-1)
    _, top_idx = jax.lax.top_k(flat, TARGET_EDGES)
    upper = jnp.zeros((N * N,), dtype=probs.dtype).at[top_idx].set(1.0).reshape(N, N)
    adj = upper + upper.T
    for _ in range(MAX_ITERS):
        deg = jnp.sum(adj, axis=1)
        nbr_probs = jnp.where(adj > 0, probs, neg_inf)
        top_vals = jax.lax.top_k(nbr_probs, MAX_DEGREE)[0]
        thresh = top_vals[:, -1]
        keep = (nbr_probs >= thresh[:, None]) | (deg <= MAX_DEGREE)[:, None]
        keep = keep.astype(adj.dtype)
        adj = adj * keep * keep.T
    return np.asarray(adj)


# ------------------------------------------------------------------- kernel
def kernel(adj_probs, label_probs, _profile=False):
    from concourse.bass_utils import run_bass_kernel_spmd

    adj_probs = np.ascontiguousarray(np.asarray(adj_probs, dtype=np.float32))
    label_probs = np.ascontiguousarray(np.asarray(label_probs, dtype=np.float32))
    assert adj_probs.shape == (N, N) and label_probs.shape == (N, C)

    labels = np.argmax(label_probs, axis=1)
    class_rows = [np.nonzero(labels == c)[0] for c in range(C)]
    sizes = np.array([len(r) for r in class_rows])
    if sizes.max() > S:
        return _reference_fallback(adj_probs, label_probs)

    prog_a, prog_c = _get_programs()

    # ---- build per-core shards
    in_maps_a = []
    core_meta = []
    for c in range(N_CORES):
        ra, rb = class_rows[2 * c], class_rows[2 * c + 1]
        na, nb = len(ra), len(rb)
        others = np.nonzero((labels != 2 * c) & (labels != 2 * c + 1))[0]
        shard = np.zeros((B, W), dtype=np.float32)
        rows_real = np.concatenate([ra, rb])
        g = adj_probs[rows_real]          # [na+nb, N] row gather
        ga = g[:na]
        gb = g[na:]
        # block columns
        shard[0:na, 0:na] = ga[:, ra]
        shard[0:na, S:S + nb] = ga[:, rb]
        shard[S:S + nb, 0:na] = gb[:, ra]
        shard[S:S + nb, S:S + nb] = gb[:, rb]
        # rest columns (packed)
        shard[0:na, B:B + len(others)] = ga[:, others]
        shard[S:S + nb, B:B + len(others)] = gb[:, others]
        # transposed own-class blocks
        blkt = np.zeros((B, S), dtype=np.float32)
        blkt[0:na, 0:na] = shard[0:na, 0:na].T
        blkt[S:S + nb, 0:nb] = shard[S:S + nb, S:S + nb].T
        in_maps_a.append({"shard": shard, "blkt": blkt})
        core_meta.append((ra, rb, na, nb))

    core_ids = list(range(N_CORES))
    res_a = run_bass_kernel_spmd(prog_a, in_maps_a, core_ids,
                                 trace=bool(_profile))

    # ---- host merge: masses, branch check, threshold
    tot = 0.0
    same_sum = 0.0
    diag_sum = 0.0
    cands = []
    for c in range(N_CORES):
        ra, rb, na, nb = core_meta[c]
        sums = res_a.results[c]["sums"].astype(np.float64)
        valid = np.zeros(B, dtype=bool)
        valid[0:na] = True
        valid[S:S + nb] = True
        sums = sums[valid]
        tot += sums[:, 0].sum() + sums[:, 1].sum() + sums[:, 2].sum()
        same_sum += sums[:, 2].sum()
        diag_sum += sums[:, 3].sum()
        cands.append(res_a.results[c]["cand"][valid])

    same_mass = same_sum - diag_sum
    diff_mass = tot - same_mass
    cur = same_mass / max(tot, 1e-30)
    # fast path requires: both weights clip-saturated and no skip branch
    if not (same_mass > 2e-6 and diff_mass > 2e-6 and 0.001 < cur < 0.09):
        return _reference_fallback(adj_probs, label_probs)

    allc = np.concatenate([c.ravel() for c in cands])
    if allc.size < 2 * K:
        return _reference_fallback(adj_probs, label_probs)
    part = np.partition(allc, allc.size - 2 * K)
    t_u = part[allc.size - 2 * K]          # 2K-th largest candidate
    if not (t_u > DIFF_U_MAX):
        return _reference_fallback(adj_probs, label_probs)

    # ---- pass C: threshold + emit
    t_arr = np.full((128, 1), t_u, dtype=np.float32)
    in_maps_c = [{"ublk": res_a.results[c]["ublk"], "thr": t_arr}
                 for c in range(N_CORES)]
    res_c = run_bass_kernel_spmd(prog_c, in_maps_c, core_ids,
                                 trace=bool(_profile))

    # ---- validity checks
    deg_total = 0.0
    deg_max = 0.0
    for c in range(N_CORES):
        ra, rb, na, nb = core_meta[c]
        deg = res_c.results[c]["deg"][:, 0]
        deg_total += float(deg[0:na].sum() + deg[S:S + nb].sum())
        if na:
            deg_max = max(deg_max, float(deg[0:na].max()))
        if nb:
            deg_max = max(deg_max, float(deg[S:S + nb].max()))
    if deg_total != 2 * K or deg_max > MAXD:
        return _reference_fallback(adj_probs, label_probs)

    # ---- assemble full output
    out = np.zeros((N, N), dtype=np.float32)
    for c in range(N_CORES):
        ra, rb, na, nb = core_meta[c]
        osh = res_c.results[c]["oshard"]
        out[np.ix_(ra, ra)] = osh[0:na, 0:na]
        out[np.ix_(rb, rb)] = osh[S:S + nb, S:S + nb]

    if _profile:
        kernel._last_profile = {
            "pass_a_ns": res_a.exec_time_ns,
            "pass_c_ns": res_c.exec_time_ns,
            "total_ns": (res_a.exec_time_ns or 0) + (res_c.exec_time_ns or 0),
        }
    return out


kernel._last_profile = None
